# revision 21
# baseline (speedup 1.0000x reference)
"""Lucas-Kanade delta_p kernel for 8 trn2 NeuronCores.

Strategy: every per-point output is derived from 15x15 box-sums of five
per-pixel product maps (Ix^2, IxIy, Iy^2, Ix*E, Iy*E with E = img2-img1).
Points lie in [0,1000)^2 so only the top-left ~1016x1016 corner matters.
Each core owns a 125-row y-band of the map domain.

Pipeline (x-chunked so the GPSIMD gather overlaps map building):
 - vertical Sobel as banded matmuls on PE (f32)
 - horizontal Sobel as shifted adds on DVE (bf16)
 - per x-chunk: bf16 products, vertical 15-box as bf16 banded matmul
   (f32 PSUM), horizontal 15-box as f32 cumulative scan + one subtract,
   per-pixel 2x2 solve on the map (reciprocal_approx_fast) -> dx,dy maps
 - per-point ap_gather with d=8 (4 x-positions x 2 channels per fetch;
   points of a 16-partition group sharing x//4 share a gather slot)
 - host-built 0/1 mask + block-diagonal matmul picks the right row out
   of each 16-partition group; result DMA'd out per chunk; host picks
   the right x%4 sub-channel per point.
Host buckets points by (band, group, x-chunk) and unpermutes the result,
so no cross-device communication is needed.
"""

import numpy as np
import ml_dtypes

import concourse.bass as bass
import concourse.bacc as bacc
import concourse.mybir as mybir
from concourse.tile import TileContext
from concourse.bass_utils import run_bass_kernel_spmd

F32 = mybir.dt.float32
BF16 = mybir.dt.bfloat16
I16 = mybir.dt.int16
NPBF = ml_dtypes.bfloat16

NCORES = 8
BAND = 125          # output map rows per core
COLS = 1024         # image columns loaded (need 0..1016)
IMG_ROWS = 144      # band image rows loaded (need 125+14+2 = 141)
PATCH = 15
NCH = 4             # x chunks
CW = 256            # map columns per chunk
PCOLS = CW + 16     # product columns per chunk (2 pad left + 14 halo right)
DU = 4              # x positions per gather slot
DD = 2 * DU         # gather d (channels per slot)


def _band_matrices():
    wsmA = np.zeros((128, 128), np.float32)   # vertical (2,4,2) main block
    wsmB = np.zeros((16, 128), np.float32)    # spill rows 128..129
    wdfA = np.zeros((128, 128), np.float32)   # vertical (2,0,-2)
    wdfB = np.zeros((16, 128), np.float32)
    sm = (2.0, 4.0, 2.0)
    df = (2.0, 0.0, -2.0)
    for m in range(128):
        for u in range(3):
            k = m + u
            if k < 128:
                wsmA[k, m] = sm[u]
                wdfA[k, m] = df[u]
            else:
                wsmB[k - 128, m] = sm[u]
                wdfB[k - 128, m] = df[u]
    wsmBB = np.zeros((16, 16), np.float32)    # P rows 128..138 from imgB
    wdfBB = np.zeros((16, 16), np.float32)
    for m in range(14):
        for u in range(3):
            k = m + u
            if k < 16:
                wsmBB[k, m] = sm[u]
                wdfBB[k, m] = df[u]
    bxA = np.zeros((128, 128), np.float32)    # vertical 15-box, main
    bxB = np.zeros((16, 128), np.float32)     # spill rows 128..138
    for m in range(BAND):
        for k in range(m, m + PATCH):
            if k < 128:
                bxA[k, m] = 1.0
            else:
                bxB[k - 128, m] = 1.0
    bdm = np.zeros((128, 8), np.float32)      # block-diag 16->1 reduce
    for p in range(128):
        bdm[p, p // 16] = 1.0
    # packed: wpA = [wsmA | wdfA | bdm] (128-part),
    #         wpB = [wsmB | wdfB | wsmBB | wdfBB] (16-part); all exact in bf16
    wpA = np.concatenate([wsmA, wdfA, bdm], axis=1).astype(NPBF)
    wpB = np.concatenate([wsmB, wdfB, wsmBB, wdfBB], axis=1).astype(NPBF)
    return dict(wpA=wpA, wpB=wpB,
                bxA=bxA.astype(NPBF), bxB=bxB.astype(NPBF))


def build_core_inputs(img1, img2, points):
    """Bucket points by (core band, 16-row group, x chunk); assign gather
    slots sharing x//DU, with at most one distinct y-row per (slot, x%DU)."""
    im1 = np.asarray(img1).reshape(img1.shape[-2], img1.shape[-1])
    im2 = np.asarray(img2).reshape(img2.shape[-2], img2.shape[-1])
    pts = np.asarray(points)
    n = pts.shape[0]
    xs = pts[:, 0].astype(np.int64)
    ys = pts[:, 1].astype(np.int64)
    core = ys // BAND
    yl = ys - core * BAND
    grp = yl // 16
    lrow = yl % 16
    chk = xs // CW
    xl = xs - chk * CW
    x4 = xl // DU
    u = xl % DU

    # slot assignment: per (core, chunk, grp, x4): slot r for the r-th
    # distinct row at each u.  Identical (x4, u, row) shares a slot+cell.
    pj = np.empty(n, np.int64)
    cells = {}
    slot_ctr = np.zeros((NCORES, NCH, 8), np.int64)
    for i in range(n):
        key = (core[i], chk[i], grp[i], x4[i])
        ck = cells.get(key)
        if ck is None:
            ck = [dict() for _ in range(DU)]
            cells[key] = ck
        d = ck[u[i]]
        r = d.get(lrow[i])
        if r is None:
            r = len(d)
            d[lrow[i]] = r
        pj[i] = r  # rank within key; global slot assigned later
    # assign global slot ids per bucket: keys in first-seen order, each key
    # occupies max-cell-count consecutive slots
    key_base = {}
    for i in range(n):
        key = (core[i], chk[i], grp[i], x4[i])
        if key not in key_base:
            c, k, g, _ = key
            nk = max(len(d) for d in cells[key])
            key_base[key] = slot_ctr[c, k, g]
            slot_ctr[c, k, g] += nk
    J = int(-(-slot_ctr.max() // 32) * 32)

    mats = _band_matrices()
    JW = J // 16
    idx_h = np.zeros((NCORES, 128, NCH * JW), np.int16)
    msk_h = np.zeros((NCORES, 128, NCH * J * DD), NPBF)
    slot_of = np.empty(n, np.int64)
    for i in range(n):
        c, k, g = core[i], chk[i], grp[i]
        j = key_base[(c, k, g, x4[i])] + pj[i]
        slot_of[i] = j
        idx_h[c, 16 * g + j % 16, k * JW + j // 16] = x4[i]
        cbase = (k * J + j) * DD + u[i] * 2
        msk_h[c, 16 * g + lrow[i], cbase] = 1.0
        msk_h[c, 16 * g + lrow[i], cbase + 1] = 1.0

    im1b = im1.astype(NPBF)
    im2b = im2.astype(NPBF)
    in_maps = []
    for c in range(NCORES):
        r0 = c * BAND
        m = dict(mats)
        m["img1b"] = np.ascontiguousarray(im1b[r0:r0 + IMG_ROWS, :COLS])
        m["img2b"] = np.ascontiguousarray(im2b[r0:r0 + IMG_ROWS, :COLS])
        m["idx"] = idx_h[c]
        m["msk"] = msk_h[c]
        in_maps.append(m)
    pick = (core, grp, chk, slot_of, u)
    return in_maps, pick, J


_prog_cache = {}


def build_program(J):
    if J in _prog_cache:
        return _prog_cache[J]
    nc = bacc.Bacc(None, target_bir_lowering=False, debug=True)
    img1b = nc.declare_dram_parameter("img1b", [IMG_ROWS, COLS], BF16, isOutput=False)
    img2b = nc.declare_dram_parameter("img2b", [IMG_ROWS, COLS], BF16, isOutput=False)
    dws = {}
    wspecs = (("wpA", [128, 264], BF16), ("wpB", [16, 288], BF16),
              ("bxA", [128, 128], BF16), ("bxB", [16, 128], BF16))
    for nm, shp, dt in wspecs:
        dws[nm] = nc.declare_dram_parameter(nm, shp, dt, isOutput=False)
    JW = J // 16
    idx = nc.declare_dram_parameter("idx", [128, NCH * JW], I16, isOutput=False)
    msk = nc.declare_dram_parameter("msk", [128, NCH * J * DD], BF16,
                                    isOutput=False)
    outr = nc.declare_dram_parameter("outr", [8, NCH * J * DD], F32,
                                     isOutput=True)

    AL = mybir.AluOpType
    AF = mybir.ActivationFunctionType
    with TileContext(nc) as tc:
        with tc.tile_pool(name="cn", bufs=1) as cn, \
             tc.tile_pool(name="pp", bufs=2) as pp, \
             tc.tile_pool(name="mp", bufs=2) as mp, \
             tc.tile_pool(name="gt", bufs=4) as gt, \
             tc.tile_pool(name="ps", bufs=4, space="PSUM") as ps:
            # ---- loads: weights first (small), then images, then idx/msk
            wts = {}
            for nm, shp, dt in wspecs:
                wt = cn.tile(shp, dt, tag=nm)
                nc.sync.dma_start(out=wt[:], in_=dws[nm][:])
                wts[nm] = wt
            wsmA = wts["wpA"][:, 0:128]
            wdfA = wts["wpA"][:, 128:256]
            bdm = wts["wpA"][:, 256:264]
            wsmB = wts["wpB"][:, 0:128]
            wdfB = wts["wpB"][:, 128:256]
            wsmBB = wts["wpB"][:, 256:272]
            wdfBB = wts["wpB"][:, 272:288]
            imgA = cn.tile([128, COLS], BF16, tag="imgA")
            imgB = cn.tile([16, COLS], BF16, tag="imgB")
            im2A = cn.tile([128, COLS], BF16, tag="im2A")
            im2B = cn.tile([16, COLS], BF16, tag="im2B")
            nc.sync.dma_start(out=imgA[:], in_=img1b[0:128, :])
            nc.sync.dma_start(out=imgB[:], in_=img1b[128:144, :])
            nc.sync.dma_start(out=im2A[:], in_=img2b[0:128, :])
            nc.sync.dma_start(out=im2B[:], in_=img2b[128:144, :])
            idxt = cn.tile([128, NCH * JW], I16, tag="idxt")
            nc.sync.dma_start(out=idxt[:], in_=idx[:])
            mt = cn.tile([128, NCH * J * DD], BF16, tag="mt")
            nc.sync.dma_start(out=mt[:], in_=msk[:])

            # ---- vertical Sobel (PE, f32) -------------------------------
            sxA = ps.tile([128, COLS], F32, tag="big")
            syA = ps.tile([128, COLS], F32, tag="big")
            sxB = ps.tile([16, COLS], F32, tag="big")
            syB = ps.tile([16, COLS], F32, tag="big")
            for c0 in range(0, COLS, 512):
                cs = slice(c0, c0 + 512)
                nc.tensor.matmul(out=sxA[:, cs], lhsT=wsmA, rhs=imgA[:, cs],
                                 start=True, stop=False)
                nc.tensor.matmul(out=sxA[:, cs], lhsT=wsmB, rhs=imgB[:, cs],
                                 start=False, stop=True)
                nc.tensor.matmul(out=syA[:, cs], lhsT=wdfA, rhs=imgA[:, cs],
                                 start=True, stop=False)
                nc.tensor.matmul(out=syA[:, cs], lhsT=wdfB, rhs=imgB[:, cs],
                                 start=False, stop=True)
                nc.tensor.matmul(out=sxB[:, cs], lhsT=wsmBB, rhs=imgB[:, cs],
                                 start=True, stop=True)
                nc.tensor.matmul(out=syB[:, cs], lhsT=wdfBB, rhs=imgB[:, cs],
                                 start=True, stop=True)

            # ---- horizontal Sobel + E (DVE, bf16) -----------------------
            grads = {}
            for tier, PP, sx, sy, i1, i2 in (
                ("A", 128, sxA, syA, imgA, im2A),
                ("B", 16, sxB, syB, imgB, im2B),
            ):
                E = cn.tile([PP, COLS], BF16, tag=f"E{tier}")
                nc.vector.tensor_tensor(out=E[:], in0=i2[:], in1=i1[:], op=AL.subtract)
                sxs = cn.tile([PP, COLS], BF16, tag=f"sxs{tier}")
                nc.scalar.copy(out=sxs[:], in_=sx[:])
                sys_ = cn.tile([PP, COLS], BF16, tag=f"sys{tier}")
                nc.scalar.copy(out=sys_[:], in_=sy[:])
                Ix = cn.tile([PP, COLS], BF16, tag=f"Ix{tier}")
                nc.vector.tensor_tensor(out=Ix[:, 0:1022], in0=sxs[:, 0:1022],
                                        in1=sxs[:, 2:1024], op=AL.subtract)
                nc.vector.memset(Ix[:, 1022:1024], 0.0)
                t1 = cn.tile([PP, COLS], BF16, tag=f"t1{tier}")
                nc.vector.tensor_tensor(out=t1[:, 0:1023], in0=sys_[:, 0:1023],
                                        in1=sys_[:, 1:1024], op=AL.add)
                Iy = cn.tile([PP, COLS], BF16, tag=f"Iy{tier}")
                nc.vector.tensor_tensor(out=Iy[:, 0:1022], in0=t1[:, 0:1022],
                                        in1=t1[:, 1:1023], op=AL.add)
                nc.vector.memset(Iy[:, 1022:1024], 0.0)
                grads[tier] = (Ix, Iy, E)

            # ---- per x-chunk: products, boxes, map-solve, gather --------
            gs = []
            for k in range(NCH):
                x0 = CW * k
                lo = x0 - 2                      # product col c <-> x = lo + c
                hi = min(lo + PCOLS, COLS)
                dst0 = 0 if k else 2             # chunk 0: cols [0,2) zeroed
                src0 = lo + dst0
                w = hi - src0
                prods = {}
                for tier, PP in (("A", 128), ("B", 16)):
                    Ix, Iy, E = grads[tier]
                    pl = []
                    for ci, (uu, vv) in enumerate(
                            ((Ix, Ix), (Ix, Iy), (Iy, Iy), (Ix, E), (Iy, E))):
                        P = pp.tile([PP, PCOLS], BF16, tag=f"P{tier}{ci}")
                        if dst0:
                            nc.vector.memset(P[:, 0:dst0], 0.0)
                        if uu is vv:
                            nc.scalar.activation(out=P[:, dst0:dst0 + w],
                                                 in_=uu[:, src0:src0 + w],
                                                 func=AF.Square)
                        else:
                            nc.vector.tensor_tensor(out=P[:, dst0:dst0 + w],
                                                    in0=uu[:, src0:src0 + w],
                                                    in1=vv[:, src0:src0 + w],
                                                    op=AL.mult)
                        if dst0 + w < PCOLS:
                            nc.vector.memset(P[:, dst0 + w:PCOLS], 0.0)
                        pl.append(P)
                    prods[tier] = pl
                maps = []
                for ci in range(5):
                    v = ps.tile([128, PCOLS], F32, tag="big")
                    nc.tensor.matmul(out=v[:], lhsT=wts["bxA"][:],
                                     rhs=prods["A"][ci][:], start=True, stop=False)
                    nc.tensor.matmul(out=v[:], lhsT=wts["bxB"][:],
                                     rhs=prods["B"][ci][:], start=False, stop=True)
                    C = mp.tile([128, PCOLS], F32, tag=f"C{ci}")
                    nc.vector.tensor_tensor_scan(out=C[:], data0=v[:], data1=C[:],
                                                 initial=0.0, op0=AL.add,
                                                 op1=AL.bypass)
                    M = mp.tile([128, CW], F32, tag=f"M{ci}")
                    nc.vector.tensor_tensor(out=M[:], in0=C[:, 16:16 + CW],
                                            in1=C[:, 1:1 + CW], op=AL.subtract)
                    maps.append(M)
                H00, H01, H11, B0, B1 = maps
                m1 = mp.tile([128, CW], F32, tag="m1")
                nc.vector.tensor_tensor(out=m1[:], in0=H00[:], in1=H11[:], op=AL.mult)
                sq = mp.tile([128, CW], F32, tag="sq")
                nc.scalar.activation(out=sq[:], in_=H01[:], func=AF.Square)
                det = mp.tile([128, CW], F32, tag="det")
                nc.vector.scalar_tensor_tensor(out=det[:], in0=m1[:], scalar=1e-30,
                                               in1=sq[:], op0=AL.add,
                                               op1=AL.subtract)
                r = mp.tile([128, CW], F32, tag="r")
                nc.vector.reciprocal_approx_fast(out=r[:], in_=det[:])
                S = gt.tile([128, CW * 2], BF16, tag="S")
                sv = S[:].rearrange("p (x c) -> p x c", c=2)
                na = mp.tile([128, CW], F32, tag="na")
                nb = mp.tile([128, CW], F32, tag="nb")
                for c2, (hA, bA, hB, bB) in enumerate(
                        ((H11, B0, H01, B1), (H00, B1, H01, B0))):
                    nc.vector.tensor_tensor(out=na[:], in0=hA[:], in1=bA[:],
                                            op=AL.mult)
                    nc.vector.tensor_tensor(out=nb[:], in0=hB[:], in1=bB[:],
                                            op=AL.mult)
                    nc.vector.tensor_tensor(out=na[:], in0=na[:], in1=nb[:],
                                            op=AL.subtract)
                    nc.vector.tensor_tensor(out=sv[:, :, c2], in0=na[:], in1=r[:],
                                            op=AL.mult)
                # ---- gather (slots share x//DU) -------------------------
                g = gt.tile([128, J * DD], BF16, tag="g")
                with tc.high_priority():
                    nc.gpsimd.ap_gather(out_ap=g[:], in_ap=S[:],
                                        idxs_ap=idxt[:, k * JW:(k + 1) * JW],
                                        channels=128, num_elems=CW // DU, d=DD,
                                        num_idxs=J)
                gs.append(g)

            # ---- select + out (after all builds: no head-of-line block) -
            for k in range(NCH):
                g = gs[k]
                nc.vector.tensor_tensor(out=g[:], in0=g[:],
                                        in1=mt[:, k * J * DD:(k + 1) * J * DD],
                                        op=AL.mult)
                dout = gt.tile([8, J * DD], F32, tag="dout")
                NF = J * DD
                for c0 in range(0, NF, 512):
                    cwi = min(512, NF - c0)
                    bps = ps.tile([8, 512], F32, tag="big")
                    nc.tensor.matmul(out=bps[:, :cwi], lhsT=bdm,
                                     rhs=g[:, c0:c0 + cwi], start=True, stop=True)
                    nc.scalar.copy(out=dout[:, c0:c0 + cwi], in_=bps[:, :cwi])
                nc.sync.dma_start(out=outr[:, k * NF:(k + 1) * NF], in_=dout[:])

    nc.compile()
    _prog_cache[J] = nc
    return nc


def _run(img1, img2, points, trace=False):
    in_maps, pick, J = build_core_inputs(img1, img2, points)
    nc = build_program(J)
    res = run_bass_kernel_spmd(nc, in_maps, list(range(NCORES)), trace=trace)
    core, grp, chk, slot, u = pick
    n = points.shape[0]
    full = np.zeros((n, 2), np.float32)
    per_core = [res.results[c]["outr"].reshape(8, NCH, J, DU, 2)
                for c in range(NCORES)]
    stack = np.stack(per_core)  # (ncores, 8, NCH, J, DU, 2)
    full[:] = stack[core, grp, chk, slot, u]
    return full, res


def kernel(img1, img2, points1):
    full, _ = _run(np.asarray(img1), np.asarray(img2), np.asarray(points1))
    return full
